# revision 45
# baseline (speedup 1.0000x reference)
"""Trainium2 Bass kernel for nn_CrystalHypergraphConv (8 NeuronCores, SPMD).

Node-contiguous edge sharding: edges sorted by destination node (idx0) and
split into 8 contiguous node ranges with ~equal edge counts, so all segment
reductions (softmax-aggregation sums) are core-local.  Only tiny stat
vectors (BatchNorm moments) and the pooled [128,128] graph matrix are
all-reduced.

Per core, edges are packed into fixed 2048-edge "windows" covering <=128
consecutive nodes, so the SPMD program is identical across cores; all
per-core variation lives in data streams.

Folding:  origin == remote in the reference, so
  z = A[idx0] + B[idx1],  A = h @ (lin_w[:H]+lin_w[2H:]),
  B = cf_raw_pad @ Wbc_pad (bembed and all biases folded, ones column).
BN1 is folded into A/Wbc after an all-reduce of moments (sum z analytic via
degree-weighted sums; sum z^2 via ACT Square accum_out).  The segment
softmax drops the max-subtraction (alpha = t*msg >= 0 so exp is safe and
denom >= 1 makes the +1e-16 negligible).

Under axon the per-call wall time is dominated by host->device input
transfer and per-call jit relowering, so v4 minimizes shipped bytes
(~183 MB -> ~14 MB) and per-call host work:
  - hedge table: one merged [TROWS, 44]-col bf16 table (row 0 = zero
    sentinel), row-sharded 1/8 per core and AllGathered on device, then
    widened to 256B rows for dma_gather; gathered cols 44..127 are garbage
    and every matmul contracts over [0:TCOLS) only.
  - gather idx streams shipped compact [16, W*128] (idow as uint8) and
    replicated on-device to the [128, .] layout dma_gather expects.
  - node features shipped int8 with per-feature scales folded into the
    embed matrix (f16 on device); pooling one-hot built on device from a
    per-slot graph-id vector; iota/identity tables built on device.
  - small weights packed into blobs (w_o|wbc|l1w|owc row-sharded +
    AllGathered); module JSON memoized so repeat calls skip ~80ms of
    re-serialization; a persistent XLA compile cache makes repeat calls
    skip neuronx-cc entirely.
"""

import os
import sys
import numpy as np

for _p in ("/opt/trn_rl_repo", "/root/.axon_site/_ro/trn_rl_repo"):
    if os.path.isdir(_p) and _p not in sys.path:
        sys.path.insert(0, _p)

import jax

# persistent XLA compilation cache: repeat executions of the same built
# program skip re-lowering through neuronx-cc (the in-memory jit cache
# can't hit because run_bass_via_pjrt makes a fresh closure per call)
jax.config.update("jax_compilation_cache_dir", "/tmp/jax_comp_cache")
jax.config.update("jax_persistent_cache_min_compile_time_secs", 0.0)
jax.config.update("jax_persistent_cache_min_entry_size_bytes", 0)

import concourse.bass as bass
import concourse.bacc as bacc
import concourse.mybir as mybir
import concourse.tile as tile
from concourse.bass_utils import run_bass_kernel_spmd
from concourse.tile_rust import add_dep_helper

import ml_dtypes

BF16 = ml_dtypes.bfloat16
FP32 = np.float32

NCORES = 8
H = 128
WIN_E = 2048
TPW = WIN_E // 128          # 16 tiles per window
IDC = WIN_E // 16           # idx stream columns per window (128)
EPS = 1e-5
NUM_GRAPHS = 128
CHUNK_W = 3
TCOLS = 44                  # shipped hedge-feature cols (41 real + pad)

AF = mybir.ActivationFunctionType
ALU = mybir.AluOpType
DT = mybir.dt
AX = mybir.AxisListType


# ---------------------------------------------------------------------------
# host-side layout
# ---------------------------------------------------------------------------

def _build_layout(idx0, idx1, N):
    E = idx0.shape[0]
    order = np.argsort(idx0, kind="stable")
    s0 = idx0[order]
    s1 = idx1[order]
    counts = np.bincount(s0, minlength=N)
    cum = np.cumsum(counts)
    node_cuts = [0]
    for c in range(1, NCORES):
        n = int(np.searchsorted(cum, E * c // NCORES))
        node_cuts.append(min(n + 1, N))
    node_cuts.append(N)

    cores = []
    for c in range(NCORES):
        n0, n1 = node_cuts[c], node_cuts[c + 1]
        n0 = min(n0, n1)
        e0 = 0 if n0 == 0 else int(cum[n0 - 1])
        e1 = int(cum[n1 - 1]) if n1 > 0 else 0
        ci0, ci1 = s0[e0:e1], s1[e0:e1]
        wins = []
        i, Ec, nstart = 0, ci0.shape[0], n0
        while i < Ec or nstart < n1:
            nmax = min(nstart + 128, n1)
            j = i
            while j < Ec and ci0[j] < nmax and (j - i) < WIN_E:
                j += 1
            if j > i and (j - i) == WIN_E and j < Ec and ci0[j] == ci0[j - 1]:
                nb = ci0[j - 1]
                while j > i and ci0[j - 1] == nb:
                    j -= 1
            nend = int(ci0[j - 1]) + 1 if j > i else nstart
            nend = min(max(nend, min(nstart + 1, n1)), nmax)
            if j == i and nend == nstart:
                nend = nmax
            wins.append((i, j, nstart, nend))
            i, nstart = j, nend
        cores.append(dict(n0=n0, n1=n1, i0=ci0, i1=ci1, wins=wins))

    W = max(len(c["wins"]) for c in cores)
    W = -(-W // CHUNK_W) * CHUNK_W
    for c in cores:
        while len(c["wins"]) < W:
            c["wins"].append((len(c["i0"]), len(c["i0"]), c["n1"], c["n1"]))
        c["W"] = W
    return cores, W


def _pack_idx16(a):
    n = a.shape[0]
    w = a.reshape(n // 16, 16).T.astype(np.int16)
    return np.ascontiguousarray(w)   # compact [16, n/16]; replicated on device


def _prep_core(core, x, hped_f32, batch, NHE, HILO, xsc):
    W = core["W"]
    NS = W * 128
    ow = np.full(W * WIN_E, -1, np.int64)
    ce = np.full(W * WIN_E, NHE, np.int64)
    slot_node = np.full(NS, -1, np.int64)
    for w, (es, ee, ns, ne) in enumerate(core["wins"]):
        cnt = ee - es
        ow[w * WIN_E: w * WIN_E + cnt] = core["i0"][es:ee] - ns
        ce[w * WIN_E: w * WIN_E + cnt] = core["i1"][es:ee]
        if ne > ns:
            slot_node[w * 128: w * 128 + (ne - ns)] = np.arange(ns, ne)
    valid = slot_node >= 0

    d = {}
    xT = np.zeros((96, NS), FP32)
    xT[:92, valid] = x[slot_node[valid]].T
    xT[92, :] = 1.0
    # int8 per-feature quantization; scales are folded into embp on host
    # (xsc is identical across cores: computed from the global x)
    d["xT"] = np.clip(np.round(xT / xsc[:, None]), -127, 127).astype(np.int8)
    # merged table: row 0 = zero sentinel, rows 1..NHE = hedge feats, NHE+1 = 0
    lo = np.where(ce < HILO, ce + 1, 0).astype(np.int16)
    hi = np.where(ce >= HILO, ce - HILO, NHE - HILO).astype(np.int16)
    d["idlo"] = _pack_idx16(lo)
    d["idhi"] = _pack_idx16(hi)
    # ow fits uint8 (slot 0..127, pad sentinel 128 -> ident's zero stripe);
    # widened to int16 on device
    d["idow"] = _pack_idx16(
        np.where(ow >= 0, ow, 128).astype(np.int16)).astype(np.uint8)
    deg = np.zeros(NS, np.float64)
    real = ow >= 0
    sl = (np.arange(W * WIN_E) // WIN_E) * 128 + np.clip(ow, 0, 127)
    np.add.at(deg, sl[real], 1.0)
    d["deg"] = np.ascontiguousarray(deg.reshape(W, 128).T).astype(BF16)
    v = hped_f32[ce[real]].astype(np.float64).sum(0)
    d["vvec"] = v.astype(BF16).reshape(128, 1)
    g = np.full(NS, -1.0, FP32)
    g[valid] = batch[slot_node[valid]].astype(FP32)
    d["gcol"] = np.ascontiguousarray(g.reshape(W, 128).T.astype(FP32))
    return d


def _host_prep(inputs):
    x = np.asarray(inputs["x"], FP32)
    hed = np.asarray(inputs["hedge_attrs"], FP32)
    iri = np.asarray(inputs["inter_relations_index"])
    batch = np.asarray(inputs["batch"]).astype(np.int64)
    N = int(inputs["num_nodes"])
    NHE = hed.shape[0]
    E = iri.shape[1]
    idx0 = np.asarray(iri[0]).astype(np.int64)
    idx1 = np.asarray(iri[1]).astype(np.int64)
    HILO = min(32767, NHE)  # low table holds rows [0, HILO)

    ew = np.asarray(inputs["embed_w"], FP32)
    eb = np.asarray(inputs["embed_b"], FP32)
    bw = np.asarray(inputs["bembed_w"], FP32)
    bb = np.asarray(inputs["bembed_b"], FP32)
    lw = np.asarray(inputs["lin_w"], FP32)
    lb = np.asarray(inputs["lin_b"], FP32)

    W_o = (lw[:H] + lw[2 * H:]).astype(FP32)
    Wbc_pad = np.zeros((128, 256), FP32)
    Wbc_pad[0] = bb @ lw[H:2 * H] + lb
    Wbc_pad[1:41] = bw @ lw[H:2 * H]

    hped = np.zeros((NHE + 1, 128), FP32)
    hped[:NHE, 0] = 1.0
    hped[:NHE, 1:41] = hed

    counts = np.bincount(batch, minlength=NUM_GRAPHS).astype(FP32)
    cinv = (1.0 / np.maximum(counts, 1.0)).astype(FP32)

    cores, W = _build_layout(idx0, idx1, N)

    xsc = np.ones(96, FP32)
    xsc[:92] = np.maximum(np.abs(x).max(axis=0), 1e-6) / 127.0
    xsc[92] = 1.0 / 127.0
    embp = np.zeros((96, 128), FP32)
    embp[:92] = ew
    embp[92] = eb
    embp *= xsc[:, None]

    # merged hedge table, sharded across cores + AllGathered on device:
    # row 0 = zero sentinel (lo side), rows 1..NHE = hped, NHE+1.. = zeros.
    # Only 48 of the 128 gather columns are shipped; cols 48..127 of the
    # device table stay garbage and all matmuls contract over [0:48) only
    # (wbc rows 41..127 are zero anyway).
    TROWS = -(-(NHE + 2) // (128 * NCORES)) * (128 * NCORES)
    tbl = np.zeros((TROWS, TCOLS), BF16)
    tbl[1:NHE + 1] = hped[:NHE, :TCOLS].astype(BF16)
    tshard = TROWS // NCORES

    # pack small constants into 3 blobs (fewer jit params per call)
    cbf = np.zeros((128, 770), BF16)          # w_o | wbc | l1w | owc
    cbf[:, 0:256] = W_o.astype(BF16)
    cbf[:, 256:512] = Wbc_pad.astype(BF16)
    cbf[:, 512:768] = np.asarray(inputs["l1_w"], FP32).astype(BF16)
    cbf[:, 768:770] = np.asarray(inputs["out_w"], FP32).reshape(2, 128).T.astype(BF16)
    crow = np.zeros((1, 896), FP32)           # bn1g|bn1b|bn2g|bn2b|cinvr
    crow[0, 0:256] = np.asarray(inputs["bn1_g"], FP32)
    crow[0, 256:512] = np.asarray(inputs["bn1_b"], FP32)
    crow[0, 512:640] = np.asarray(inputs["bn2_g"], FP32)
    crow[0, 640:768] = np.asarray(inputs["bn2_b"], FP32)
    crow[0, 768:896] = cinv
    cf2 = np.zeros((128, 3), FP32)            # tagg | l1bc
    cf2[:, 0] = float(np.asarray(inputs["aggr_t"]))
    cf2[:, 1:3] = np.asarray(inputs["l1_b"], FP32).reshape(2, 128).T

    shared = {
        "embp": embp.astype(np.float16),
        "crow": np.ascontiguousarray(crow),
        "cf2": np.ascontiguousarray(cf2),
    }
    outb = float(np.asarray(inputs["out_b"]).reshape(-1)[0])

    per_core = [_prep_core(c, x, hped, batch, NHE, HILO, xsc) for c in cores]
    for ci, d in enumerate(per_core):
        d["tbls"] = np.ascontiguousarray(tbl[ci * tshard:(ci + 1) * tshard])
        d["cbfs"] = np.ascontiguousarray(cbf[ci * 16:(ci + 1) * 16])
        d["pbf"] = np.ascontiguousarray(
            np.hstack([d.pop("deg"), d.pop("vvec")]))
    meta = dict(W=W, E=E, N=N, NHE=NHE, outb=outb, TROWS=TROWS, HILO=HILO)
    return shared, per_core, meta


# ---------------------------------------------------------------------------
# device program
# ---------------------------------------------------------------------------

_NPDT = {np.dtype(FP32): DT.float32, np.dtype(BF16): DT.bfloat16,
         np.dtype(np.int16): DT.int16, np.dtype(np.float16): DT.float16,
         np.dtype(np.int8): DT.int8, np.dtype(np.uint8): DT.uint8}


def build_program(shared, core0, meta):
    nc = bacc.Bacc(None)
    t_in = {}
    for name, arr in {**shared, **core0}.items():
        t_in[name] = nc.dram_tensor(name, list(arr.shape), _NPDT[arr.dtype],
                                    kind="ExternalInput")
    y_out = nc.dram_tensor("y", [1, 128], DT.float32, kind="ExternalOutput")
    with tile.TileContext(nc) as tc:
        _emit(tc, nc, t_in, y_out, meta)
    nc.compile()
    # memoize: the jit lowering serializes the module to JSON on every
    # run_bass_kernel_spmd call (~77ms for this program); it is immutable
    # after compile
    jb = nc.to_json_bytes()
    nc.to_json_bytes = lambda: jb
    return nc


def _emit(tc, nc, t_in, y_out, meta):
    f32, bf16, i16 = DT.float32, DT.bfloat16, DT.int16
    W, E, N, outb = meta["W"], meta["E"], meta["N"], meta["outb"]
    TROWS, HILO = meta["TROWS"], meta["HILO"]
    NS = W * 128
    NCH = W // CHUNK_W
    RG = [list(range(NCORES))]

    cpool = tc.alloc_tile_pool(name="const", bufs=1)

    def lc(name):
        tt = t_in[name]
        tl = cpool.tile(list(tt.shape), tt.dtype, tag=name)
        nc.sync.dma_start(tl[:], tt[:])
        return tl

    embp, gcol = lc("embp"), lc("gcol")
    crow, cf2, pbf = lc("crow"), lc("cf2"), lc("pbf")

    # weight blob: AllGather the 16-row per-core shards, then SBUF-load
    dpool = tc.alloc_tile_pool(name="dramp", bufs=1, space="DRAM")
    cbf_in = dpool.tile([16, 770], bf16, tag="cbfi")
    nc.sync.dma_start(cbf_in[:], t_in["cbfs"][:])
    cbf_dr = dpool.tile([128, 770], bf16, tag="cbfg")
    nc.gpsimd.collective_compute("AllGather", ALU.bypass, replica_groups=RG,
                                 ins=[cbf_in[:].opt()], outs=[cbf_dr[:].opt()])
    cbf = cpool.tile([128, 770], bf16, tag="cbf")
    nc.sync.dma_start(cbf[:], cbf_dr[:])

    def unpack(tg, src, c0, c1, dt_, p=128):
        tl = cpool.tile([p, c1 - c0], dt_, tag=tg)
        nc.vector.tensor_copy(tl[:], src[0:p, c0:c1])
        return tl

    w_o = unpack("w_o", cbf, 0, 256, bf16)
    wbc = unpack("wbc", cbf, 256, 512, bf16)
    l1w = unpack("l1w", cbf, 512, 768, bf16)
    owc = unpack("owc", cbf, 768, 770, bf16)
    bn1g = unpack("bn1g", crow, 0, 256, f32, p=1)
    bn1b = unpack("bn1b", crow, 256, 512, f32, p=1)
    bn2g = unpack("bn2g", crow, 512, 640, f32, p=1)
    bn2b = unpack("bn2b", crow, 640, 768, f32, p=1)
    cinvr = unpack("cinvr", crow, 768, 896, f32, p=1)
    tagg = unpack("tagg", cf2, 0, 1, f32)
    l1bc = unpack("l1bc", cf2, 1, 3, f32)
    deg = unpack("deg", pbf, 0, W, bf16)
    vvec = unpack("vvec", pbf, W, W + 1, bf16)

    # on-device constants: iota row/col, identity tables
    ioti = cpool.tile([128, 128], DT.int32)
    nc.gpsimd.iota(ioti[:], pattern=[[1, 128]], base=0, channel_multiplier=0)
    pcoli = cpool.tile([128, 1], DT.int32)
    nc.gpsimd.iota(pcoli[:], pattern=[[1, 1]], base=0, channel_multiplier=1)
    iotab = cpool.tile([128, 128], bf16)
    nc.vector.tensor_copy(iotab[:], ioti[:])
    pcolf = cpool.tile([128, 1], f32)
    nc.vector.tensor_copy(pcolf[:], pcoli[:])
    ident = cpool.tile([128, 256], bf16)
    nc.vector.memset(ident[:], 0.0)
    nc.vector.tensor_scalar(ident[:, 0:128], iotab[:], pcolf[:, 0:1],
                            None, ALU.is_equal)
    eyef = cpool.tile([128, 128], f32)
    nc.vector.tensor_scalar(eyef[:], iotab[:], pcolf[:, 0:1], None, ALU.is_equal)

    # hedge-feature table: AllGather per-core row shards of the compact
    # [TROWS, TCOLS] table, then widen rows to 256B for dma_gather
    tblc_dr = dpool.tile([TROWS, TCOLS], bf16, tag="tblc")
    tbl_in = dpool.tile([TROWS // NCORES, TCOLS], bf16, tag="tbli")
    nc.sync.dma_start(tbl_in[:], t_in["tbls"][:])
    nc.gpsimd.collective_compute("AllGather", ALU.bypass, replica_groups=RG,
                                 ins=[tbl_in[:].opt()],
                                 outs=[tblc_dr[:].opt()])
    tbl_dr = dpool.tile([TROWS, 128], bf16, tag="tbl")
    nc.sync.dma_start(tbl_dr[:, 0:TCOLS], tblc_dr[:])
    # expand compact [16, W*128] idx streams to the replicated [128, W*128]
    # layout dma_gather expects (8 copies across partition groups)
    idx_dr = {}
    with tc.tile_pool(name="idcvt", bufs=1) as icp:
        ow8 = icp.tile([16, NS], DT.uint8, tag="ow8")
        nc.sync.dma_start(ow8[:], t_in["idow"][:])
        ow16 = icp.tile([16, NS], i16, tag="ow16")
        nc.vector.tensor_copy(ow16[:], ow8[:])
        for s in ("idow", "idlo", "idhi"):
            idx_x = dpool.tile([128, NS], i16, tag=s + "_x")
            for k in range(NCORES):
                if s == "idow":
                    nc.sync.dma_start(idx_x[k * 16:(k + 1) * 16, :], ow16[:])
                else:
                    nc.sync.dma_start(idx_x[k * 16:(k + 1) * 16, :], t_in[s][:])
            idx_dr[s] = idx_x
    gsrc = dict(idx_dr=idx_dr, tbl_lo=tbl_dr[0:HILO + 1, :],
                tbl_hi=tbl_dr[HILO + 1:TROWS, :])

    # owcol[p, c] = ow of edge 128c+p, derived from the idow stream:
    # expanded idow[16k+i, j] = ow[16j+i]  →  owcol[16k+i, c] = idow[16k+i, 8c+k]
    owi = cpool.tile([128, W * TPW], i16)
    idv = idx_dr["idow"][:].rearrange("p (c e) -> p c e", e=8)
    for k in range(NCORES):
        nc.sync.dma_start(owi[k * 16:(k + 1) * 16, :],
                          idv[k * 16:(k + 1) * 16, :, k:k + 1])
    owcol = cpool.tile([128, W * TPW], f32)
    nc.vector.tensor_copy(owcol[:], owi[:])

    c_eps = cpool.tile([128, 1], f32)
    nc.vector.memset(c_eps[:], EPS)
    c_tiny = cpool.tile([128, 1], f32)
    nc.vector.memset(c_tiny[:], 1e-16)
    A_sb = cpool.tile([128, W * 256], bf16)
    wbc2 = cpool.tile([128, 256], bf16)
    stat1 = cpool.tile([1, 512], f32)

    hsl_dr = nc.dram_tensor("hsl_dr", [128, NS], DT.bfloat16)
    seg_dr = nc.dram_tensor("seg_dr", [128, W * 256], DT.float32)

    # ---------------- setup ----------------
    with tc.tile_pool(name="xt", bufs=1) as xpool, \
         tc.tile_pool(name="ht", bufs=1) as hpool, \
         tc.tile_pool(name="htmp", bufs=2) as htp, \
         tc.tile_pool(name="spsum", bufs=2, space="PSUM") as spp:
        xq = xpool.tile([96, NS], DT.int8, tag="xq")
        nc.sync.dma_start(xq[:], t_in["xT"][:])
        xT = xpool.tile([96, NS], DT.float16, tag="xf")
        nc.vector.tensor_copy(xT[:], xq[:])
        hT = hpool.tile([128, NS], bf16)
        nblk = (NS + 511) // 512
        for j in range(nblk):
            c0, c1 = j * 512, min(NS, j * 512 + 512)
            ps = spp.tile([128, 512], f32, tag="ps")
            nc.tensor.matmul(ps[:, :c1 - c0], embp[:93, :], xT[:93, c0:c1],
                             start=True, stop=True)
            nc.vector.tensor_copy(hT[:, c0:c1], ps[:, :c1 - c0])
        for w in range(W):
            ps = spp.tile([128, 128], f32, tag="ps2")
            nc.tensor.matmul(ps[:], xT[:93, w * 128:(w + 1) * 128], embp[:93, :],
                             start=True, stop=True)
            ht_ = htp.tile([128, 128], bf16, tag="hw")
            nc.vector.tensor_copy(ht_[:], ps[:])
            nc.sync.dma_start(hsl_dr[:, w * 128:(w + 1) * 128], ht_[:])
        for w in range(W):
            ps = spp.tile([128, 256], f32, tag="ps3")
            nc.tensor.matmul(ps[:], hT[:, w * 128:(w + 1) * 128], w_o[:],
                             start=True, stop=True)
            nc.vector.tensor_copy(A_sb[:, w * 256:(w + 1) * 256], ps[:])

    # ---------------- pass 1: BN1 stats ----------------
    with tc.tile_pool(name="g1", bufs=2) as gp, \
         tc.tile_pool(name="gi1", bufs=3) as gip, \
         tc.tile_pool(name="scr", bufs=3) as scrp, \
         tc.tile_pool(name="sz", bufs=1) as szp, \
         tc.tile_pool(name="zps", bufs=3, space="PSUM") as zpp, \
         tc.tile_pool(name="szps", bufs=1, space="PSUM") as szpp:

        sz2 = szp.tile([128, 4 * W], f32)
        psz = szpp.tile([1, 256], f32, tag="psz")

        for w in range(W):
            ie, lo, hi = _gather_window(nc, gp, gip, gsrc, ident, w)
            _pass1_window(nc, w, W, ie, lo, hi, A_sb, wbc, zpp, scrp, sz2, psz, deg)
        nc.tensor.matmul(psz[:], vvec[:], wbc[:], start=False, stop=True)
        nc.vector.tensor_copy(stat1[0:1, 0:256], psz[:])

        sz2r = szp.tile([128, 2], f32, tag="sz2r")
        for h in range(2):
            nc.vector.tensor_reduce(sz2r[:, h:h + 1], sz2[:, h * 2 * W:(h + 1) * 2 * W],
                                    AX.X, ALU.add)
        for h in range(2):
            pt = szpp.tile([1, 128], f32, tag="pt")
            nc.tensor.transpose(pt[:], sz2r[:, h:h + 1], eyef[:])
            nc.vector.tensor_copy(stat1[0:1, 256 + h * 128: 256 + (h + 1) * 128], pt[:])

    # ---------------- AR1 + BN1 fold ----------------
    with tc.tile_pool(name="ardr", bufs=1, space="DRAM") as drp, \
         tc.tile_pool(name="arsb", bufs=1) as arp:
        arin = drp.tile([1, 512], f32, tag="a1i")
        arout = drp.tile([1, 512], f32, tag="a1o")
        nc.sync.dma_start(arin[:], stat1[:])
        nc.gpsimd.collective_compute("AllReduce", ALU.add, replica_groups=RG,
                                     ins=[arin[:].opt()], outs=[arout[:].opt()])
        stat1g = arp.tile([1, 512], f32)
        nc.sync.dma_start(stat1g[:], arout[:])

        mu = arp.tile([1, 256], f32)
        var = arp.tile([1, 256], f32)
        s1 = arp.tile([1, 256], f32)
        t1 = arp.tile([1, 256], f32)
        tmp = arp.tile([1, 256], f32)
        nc.vector.tensor_scalar(mu[:], stat1g[0:1, 0:256], 1.0 / E, None, ALU.mult)
        nc.vector.tensor_scalar(var[:], stat1g[0:1, 256:512], 1.0 / E, None, ALU.mult)
        nc.vector.tensor_tensor(tmp[:], mu[:], mu[:], ALU.mult)
        nc.vector.tensor_tensor(var[:], var[:], tmp[:], ALU.subtract)
        nc.scalar.activation(tmp[:], var[:], AF.Ln, bias=c_eps[0:1, :])
        nc.scalar.activation(s1[:], tmp[:], AF.Exp, scale=-0.5)
        nc.vector.tensor_tensor(s1[:], s1[:], bn1g[:], ALU.mult)
        nc.vector.tensor_tensor(tmp[:], mu[:], s1[:], ALU.mult)
        nc.vector.tensor_tensor(t1[:], bn1b[:], tmp[:], ALU.subtract)
        s1b = arp.tile([128, 256], f32)
        t1b = arp.tile([128, 256], f32)
        nc.gpsimd.partition_broadcast(s1b[:], s1[:])
        nc.gpsimd.partition_broadcast(t1b[:], t1[:])
        for w in range(W):
            nc.vector.tensor_tensor(A_sb[:, w * 256:(w + 1) * 256],
                                    A_sb[:, w * 256:(w + 1) * 256], s1b[:], ALU.mult)
        nc.vector.tensor_tensor(wbc2[:], wbc[:], s1b[:], ALU.mult)
        nc.vector.tensor_tensor(wbc2[0:1, :], wbc2[0:1, :], t1b[0:1, :], ALU.add)

    # ---------------- pass 2 ----------------
    CH_E = CHUNK_W * WIN_E
    CH_T = CHUNK_W * TPW
    with tc.tile_pool(name="g2", bufs=2) as gp2, \
         tc.tile_pool(name="gi2", bufs=3) as gip2, \
         tc.tile_pool(name="zsb", bufs=2) as zp, \
         tc.tile_pool(name="hlf", bufs=1) as hp, \
         tc.tile_pool(name="emp", bufs=1) as emp, \
         tc.tile_pool(name="ird", bufs=4) as irp, \
         tc.tile_pool(name="sev", bufs=1) as sevp, \
         tc.tile_pool(name="zps2", bufs=3, space="PSUM") as zpp2, \
         tc.tile_pool(name="sgps", bufs=2, space="PSUM") as sgp:
        prev_e = None
        for ch in range(NCH):
            z_sb = zp.tile([128, CH_T * 256], bf16, tag="z")
            for wi in range(CHUNK_W):
                w = ch * CHUNK_W + wi
                ie, lo, hi = _gather_window(nc, gp2, gip2, gsrc, ident, w)
                aw2 = A_sb[:, w * 256:(w + 1) * 256]
                for q in range(4):
                    ztp = zpp2.tile([128, 1024], f32, tag="z2")
                    for t4 in range(4):
                        tl = q * 4 + t4
                        esl = slice(tl * 128, (tl + 1) * 128)
                        osl = slice(t4 * 256, (t4 + 1) * 256)
                        nc.tensor.matmul(ztp[:, osl], ie[:, 0, esl], aw2,
                                         start=True, stop=False)
                        nc.tensor.matmul(ztp[:, osl], lo[0:TCOLS, 0, esl],
                                         wbc2[0:TCOLS, :], start=False, stop=False)
                        nc.tensor.matmul(ztp[:, osl], hi[0:TCOLS, 0, esl],
                                         wbc2[0:TCOLS, :], start=False, stop=True)
                    o0 = (wi * TPW + q * 4) * 256
                    nc.vector.tensor_copy(z_sb[:, o0:o0 + 1024], ztp[:])
            zv = z_sb[:].rearrange("p (t c) -> p t c", c=256)
            sg = hp.tile([128, CH_E], bf16, tag="sg")
            u = hp.tile([128, CH_E], bf16, tag="u")
            sp = hp.tile([128, CH_E], bf16, tag="sp")
            msg = hp.tile([128, CH_E], bf16, tag="msg")
            em = emp.tile([128, CH_T * 256], bf16, tag="em")
            sgv = sg[:].rearrange("p (t c) -> p t c", c=128)
            uv = u[:].rearrange("p (t c) -> p t c", c=128)
            msgv = msg[:].rearrange("p (t c) -> p t c", c=128)
            emv = em[:].rearrange("p (t c) -> p t c", c=256)
            nc.scalar.activation(sgv[:], zv[:, :, 0:128], AF.Sigmoid)
            nc.scalar.activation(uv[:], zv[:, :, 128:256], AF.Exp)
            nc.scalar.activation(sp[:], u[:], AF.Ln, bias=1.0)
            nc.gpsimd.tensor_tensor(msg[:], sg[:], sp[:], ALU.mult)
            nc.scalar.activation(emv[:, :, 0:128], msgv[:], AF.Exp, scale=tagg[:])
            nc.vector.tensor_tensor(emv[:, :, 128:256], msgv[:],
                                    emv[:, :, 0:128], ALU.mult)
            for wi in range(CHUNK_W):
                w = ch * CHUNK_W + wi
                sps = sgp.tile([128, 256], f32, tag="seg")
                for t in range(TPW):
                    ird = irp.tile([128, 128], bf16, tag="ird")
                    nc.vector.tensor_scalar(
                        ird[:], iotab[:], owcol[:, w * TPW + t: w * TPW + t + 1],
                        None, ALU.is_equal)
                    esl = slice((wi * TPW + t) * 256, (wi * TPW + t + 1) * 256)
                    nc.tensor.matmul(sps[:], ird[:], em[:, esl],
                                     start=(t == 0), stop=(t == TPW - 1))
                sev = sevp.tile([128, 256], f32, tag="sev")
                nc.vector.tensor_copy(sev[:], sps[:])
                nc.sync.dma_start(seg_dr[:, w * 256:(w + 1) * 256], sev[:])

    # ---------------- tail ----------------
    with tc.tile_pool(name="tl", bufs=1) as tp, \
         tc.tile_pool(name="tps", bufs=1, space="PSUM") as tpp, \
         tc.tile_pool(name="tdr", bufs=1, space="DRAM") as tdr:
        segt = tp.tile([128, W * 256], f32, tag="segwt")
        nc.sync.dma_start(segt[:], seg_dr[:])
        segv = segt[:].rearrange("p (w c) -> p w c", c=256)

        rec = tp.tile([128, NS], f32, tag="rec")
        recv = rec[:].rearrange("p (w c) -> p w c", c=128)
        nc.scalar.activation(recv[:], segv[:, :, 0:128], AF.Ln, bias=c_tiny[:])
        nc.scalar.activation(rec[:], rec[:], AF.Exp, scale=-1.0)
        outa = tp.tile([128, NS], bf16, tag="outa")
        outav = outa[:].rearrange("p (w c) -> p w c", c=128)
        nc.vector.tensor_tensor(outav[:], recv[:], segv[:, :, 128:256], ALU.mult)
        o2 = tp.tile([128, NS], bf16, tag="o2hsl")
        nc.scalar.activation(o2[:], outa[:], AF.Square)
        ones = tp.tile([128, 1], bf16, tag="ones")
        nc.vector.memset(ones[:], 1.0)
        pso = tpp.tile([1, 128], f32, tag="pso")
        pso2 = tpp.tile([1, 128], f32, tag="pso2")
        for w in range(W):
            nc.tensor.matmul(pso[:], ones[:], outa[:, w * 128:(w + 1) * 128],
                             start=(w == 0), stop=(w == W - 1))
        for w in range(W):
            nc.tensor.matmul(pso2[:], ones[:], o2[:, w * 128:(w + 1) * 128],
                             start=(w == 0), stop=(w == W - 1))
        stat2 = tp.tile([1, 256], f32, tag="st2")
        nc.vector.tensor_copy(stat2[0:1, 0:128], pso[:])
        nc.vector.tensor_copy(stat2[0:1, 128:256], pso2[:])
        arin2 = tdr.tile([1, 256], f32, tag="a2i")
        arout2 = tdr.tile([1, 256], f32, tag="a2o")
        nc.sync.dma_start(arin2[:], stat2[:])
        nc.gpsimd.collective_compute("AllReduce", ALU.add, replica_groups=RG,
                                     ins=[arin2[:].opt()], outs=[arout2[:].opt()])
        stat2g = tp.tile([1, 256], f32, tag="st2g")
        nc.sync.dma_start(stat2g[:], arout2[:])
        mu2 = tp.tile([1, 128], f32, tag="mu2")
        var2 = tp.tile([1, 128], f32, tag="var2")
        s2 = tp.tile([1, 128], f32, tag="s2")
        t2 = tp.tile([1, 128], f32, tag="t2")
        tm2 = tp.tile([1, 128], f32, tag="tm2")
        nc.vector.tensor_scalar(mu2[:], stat2g[0:1, 0:128], 1.0 / N, None, ALU.mult)
        nc.vector.tensor_scalar(var2[:], stat2g[0:1, 128:256], 1.0 / N, None, ALU.mult)
        nc.vector.tensor_tensor(tm2[:], mu2[:], mu2[:], ALU.mult)
        nc.vector.tensor_tensor(var2[:], var2[:], tm2[:], ALU.subtract)
        nc.scalar.activation(tm2[:], var2[:], AF.Ln, bias=c_eps[0:1, :])
        nc.scalar.activation(s2[:], tm2[:], AF.Exp, scale=-0.5)
        nc.vector.tensor_tensor(s2[:], s2[:], bn2g[:], ALU.mult)
        nc.vector.tensor_tensor(tm2[:], mu2[:], s2[:], ALU.mult)
        nc.vector.tensor_tensor(t2[:], bn2b[:], tm2[:], ALU.subtract)
        s2b = tp.tile([128, 128], f32, tag="s2b")
        t2b = tp.tile([128, 128], f32, tag="t2b")
        nc.gpsimd.partition_broadcast(s2b[:], s2[:])
        nc.gpsimd.partition_broadcast(t2b[:], t2[:])
        hsl = tp.tile([128, NS], bf16, tag="o2hsl")
        nc.sync.dma_start(hsl[:], hsl_dr[:])
        wt = tp.tile([128, NS], f32, tag="segwt")
        for w in range(W):
            sl = slice(w * 128, (w + 1) * 128)
            nc.vector.tensor_tensor(wt[:, sl], outa[:, sl], s2b[:], ALU.mult)
        for w in range(W):
            sl = slice(w * 128, (w + 1) * 128)
            nc.vector.tensor_tensor(wt[:, sl], wt[:, sl], t2b[:], ALU.add)
        nc.vector.tensor_tensor(wt[:], wt[:], hsl[:], ALU.add)
        u3 = tp.tile([128, NS], bf16, tag="o2hsl")
        hf = tp.tile([128, NS], bf16, tag="hf")
        nc.scalar.activation(u3[:], wt[:], AF.Exp)
        nc.scalar.activation(hf[:], u3[:], AF.Ln, bias=1.0)
        psp = tpp.tile([128, 128], f32, tag="psp")
        for w in range(W):
            ohw = tp.tile([128, 128], bf16, tag="ohw")
            nc.vector.tensor_scalar(ohw[:], iotab[:], gcol[:, w:w + 1],
                                    None, ALU.is_equal)
            nc.tensor.matmul(psp[:], hf[:, w * 128:(w + 1) * 128], ohw[:],
                             start=(w == 0), stop=(w == W - 1))
        pool_sb = tp.tile([128, 128], f32, tag="pool")
        nc.vector.tensor_copy(pool_sb[:], psp[:])
        arin3 = tdr.tile([128, 128], f32, tag="a3i")
        arout3 = tdr.tile([128, 128], f32, tag="a3o")
        nc.sync.dma_start(arin3[:], pool_sb[:])
        nc.gpsimd.collective_compute("AllReduce", ALU.add, replica_groups=RG,
                                     ins=[arin3[:].opt()], outs=[arout3[:].opt()])
        pmf = tp.tile([128, 128], f32, tag="pmf")
        nc.sync.dma_start(pmf[:], arout3[:])
        cinvb = tp.tile([128, 128], f32, tag="cinvb")
        nc.gpsimd.partition_broadcast(cinvb[:], cinvr[:])
        nc.vector.tensor_tensor(pmf[:], pmf[:], cinvb[:], ALU.mult)
        pmT = tp.tile([128, 128], bf16, tag="pmT")
        nc.vector.tensor_copy(pmT[:], pmf[:])
        psy = tpp.tile([1, 128], f32, tag="psy")
        for hh in range(2):
            ps1 = tpp.tile([128, 128], f32, tag="l1ps")
            nc.tensor.matmul(ps1[:], l1w[:, hh * 128:(hh + 1) * 128], pmT[:],
                             start=True, stop=True)
            g1u = tp.tile([128, 128], bf16, tag="g1u")
            g1h = tp.tile([128, 128], bf16, tag="g1h")
            nc.scalar.activation(g1u[:], ps1[:], AF.Exp, bias=l1bc[:, hh:hh + 1])
            nc.scalar.activation(g1h[:], g1u[:], AF.Ln, bias=1.0)
            nc.tensor.matmul(psy[:], owc[:, hh:hh + 1], g1h[:],
                             start=(hh == 0), stop=(hh == 1))
        ysb = tp.tile([1, 128], f32, tag="ysb")
        nc.vector.tensor_scalar(ysb[:], psy[:], outb, None, ALU.add)
        nc.sync.dma_start(y_out[:], ysb[:])
    cpool.release()


def _pass1_window(nc, w, W, ie, lo, hi, A_sb, wbc, zpp, scrp, sz2, psz, deg):
    f32, bf16 = DT.float32, DT.bfloat16
    AFq = AF.Square
    for h in range(2):
        aw = A_sb[:, w * 256 + h * 128: w * 256 + (h + 1) * 128]
        wb = wbc[0:TCOLS, h * 128:(h + 1) * 128]
        for g in range(2):
            zt = zpp.tile([128, 1024], f32, tag="zt")
            for q in range(2):
                sl = slice(q * 512, (q + 1) * 512)
                esl = slice(g * 1024 + q * 512, g * 1024 + (q + 1) * 512)
                nc.tensor.matmul(zt[:, sl], aw, ie[:, 0, esl], start=True, stop=False)
                nc.tensor.matmul(zt[:, sl], wb, lo[0:TCOLS, 0, esl], start=False, stop=False)
                nc.tensor.matmul(zt[:, sl], wb, hi[0:TCOLS, 0, esl], start=False, stop=True)
            scr = scrp.tile([128, 1024], bf16, tag="scr")
            k = h * 2 * W + w * 2 + g
            nc.scalar.activation(scr[:], zt[:], AFq, accum_out=sz2[:, k:k + 1])
    nc.tensor.matmul(psz[:], deg[:, w:w + 1], A_sb[:, w * 256:(w + 1) * 256],
                     start=(w == 0), stop=False)


def _gather_window(nc, gp, gip, gsrc, ident, w):
    """Load idx streams for window w + issue the three gathers."""
    bf16, i16 = DT.bfloat16, DT.int16
    idx_dr = gsrc["idx_dr"]
    xo = gip.tile([128, IDC], i16, tag="xo")
    xl = gip.tile([128, IDC], i16, tag="xl")
    xh = gip.tile([128, IDC], i16, tag="xh")
    nc.sync.dma_start(xo[:], idx_dr["idow"][:, w * IDC:(w + 1) * IDC])
    nc.sync.dma_start(xl[:], idx_dr["idlo"][:, w * IDC:(w + 1) * IDC])
    nc.sync.dma_start(xh[:], idx_dr["idhi"][:, w * IDC:(w + 1) * IDC])
    ie = gp.tile([128, 1, WIN_E], bf16, tag="ie")
    lo = gp.tile([128, 1, WIN_E], bf16, tag="lo")
    hi = gp.tile([128, 1, WIN_E], bf16, tag="hi")
    nc.gpsimd.dma_gather(ie[:], ident[:], xo[:], WIN_E, WIN_E, 128,
                         transpose=True, sbuf_tokens_per_rank=128,
                         sbuf_free_dim_per_rank=256, single_packet=False)
    nc.gpsimd.dma_gather(lo[:], gsrc["tbl_lo"], xl[:], WIN_E, WIN_E, 128,
                         transpose=True, single_packet=False)
    nc.gpsimd.dma_gather(hi[:], gsrc["tbl_hi"], xh[:], WIN_E, WIN_E, 128,
                         transpose=True, single_packet=False)
    return ie, lo, hi


# ---------------------------------------------------------------------------
# entry point
# ---------------------------------------------------------------------------

def kernel(**inputs) -> np.ndarray:
    shared, per_core, meta = _host_prep(inputs)
    nc = build_program(shared, per_core[0], meta)
    in_maps = []
    for c in range(NCORES):
        m = {k: np.ascontiguousarray(v) for k, v in shared.items()}
        m.update({k: np.ascontiguousarray(v) for k, v in per_core[c].items()})
        in_maps.append(m)
    res = run_bass_kernel_spmd(nc, in_maps, list(range(NCORES)))
    y = np.asarray(res.results[0]["y"]).reshape(128, 1).astype(np.float32)
    return y

